# revision 9
# baseline (speedup 1.0000x reference)
"""GCN layer (message passing) on 8 Trainium2 NeuronCores.

out = relu(((D^-1/2 A D^-1/2) X) @ W.T) + X

v2 strategy (bf16 datapath, dst-sharded):
  - Destination nodes sharded across 8 cores (12500 each). Host bakes
    norm[src] into a bf16 feature table (so the one-hot S is binary) and
    applies norm[dst] as the per-partition scale of the final ReLU.
  - Edges sorted by (tile-group of 4 dst tiles, src bucket, dst tile, src).
    One dma_gather per (group, bucket) -> ~100 calls/core instead of 392,
    cutting the serial Q7 SWDGE descriptor-generation time.
  - Per 128-edge chunk: S[e, d] = (ld_e == d) built on DVE (tensor_scalar
    is_equal, bf16) or ACT (two activations, fp32->bf16), alternating to
    balance engines; PE accumulates zT[i, d] += X_c[e, i].T @ S_c[e, d]
    in fp32 PSUM from bf16 operands (no fp32 LOW/HIGH matmul split).
  - Per dst tile: zT -> bf16 SBUF copy, y = zT.T @ W.T on PE,
    relu(norm_dst * y) on ACT (per-partition scale), +residual on DVE.
  - SPMD static sizes: per (group, bucket, tile) segment = max count over
    the 8 cores; pad slots gather row 0 of the bucket and carry ld = -1
    (zero one-hot row). Unwritten tail slots of the last chunk of each
    gather call are memzeroed so 0 * garbage cannot poison PSUM.
"""

import math

import numpy as np
from ml_dtypes import bfloat16

import concourse.bacc as bacc
import concourse.mybir as mybir
from concourse.bass_utils import run_bass_kernel_spmd
from concourse.tile import TileContext

P = 128
N_CORES = 8
BUCKET_MAX = 25000  # int16 gather indices: bucket the node space
GROUP_TILES = 5  # dst tiles per gather group

N_NODES = 100000
NPC = N_NODES // N_CORES  # 12500
N_TILES = math.ceil(NPC / P)  # 98
ROWS_LAST = NPC - (N_TILES - 1) * P  # 84
NB = math.ceil(N_NODES / BUCKET_MAX)  # 4
N_GROUPS = math.ceil(N_TILES / GROUP_TILES)  # 25

RUN_K = 8  # chunks per grouped DVE S-build
ACT_EVERY = 10  # every ACT_EVERY-th run goes to the ACT 2-op path


def _prepare(features, W, edge_src, edge_dst):
    features = np.asarray(features, dtype=np.float32)
    W = np.asarray(W, dtype=np.float32)
    edge_src = np.asarray(edge_src, dtype=np.int32)
    edge_dst = np.asarray(edge_dst, dtype=np.int32)

    n_nodes, d = features.shape
    assert d == P and n_nodes == N_NODES

    degs = np.bincount(edge_dst, minlength=n_nodes).astype(np.float32)
    norm = 1.0 / np.sqrt(np.maximum(degs, 1.0), dtype=np.float32)
    table = (features * norm[:, None]).astype(bfloat16)  # norm[src] baked in

    core_of = edge_dst // NPC

    # tiles per group (last group may be short)
    gtiles = [
        list(range(g * GROUP_TILES, min((g + 1) * GROUP_TILES, N_TILES)))
        for g in range(N_GROUPS)
    ]

    # per-core sorted edges and per-(group,bucket,tile) counts
    per_core = []
    counts = np.zeros((N_CORES, N_GROUPS, NB, GROUP_TILES), np.int64)
    for k in range(N_CORES):
        sel = np.flatnonzero(core_of == k)
        src_k = edge_src[sel]
        ldst = edge_dst[sel] - k * NPC
        tile = ldst >> 7
        grp = tile // GROUP_TILES
        tin = tile % GROUP_TILES  # tile index within group
        bkt = src_k // BUCKET_MAX
        order = np.lexsort((src_k, tin, bkt, grp))
        src_s = src_k[order]
        grp_s, bkt_s, tin_s = grp[order], bkt[order], tin[order]
        ld_s = (ldst[order] & 127).astype(np.float32)
        gid = (grp_s * NB + bkt_s) * GROUP_TILES + tin_s
        cnt = np.bincount(gid, minlength=N_GROUPS * NB * GROUP_TILES)
        counts[k] = cnt.reshape(N_GROUPS, NB, GROUP_TILES)
        per_core.append((src_s, bkt_s, gid, ld_s))

    seg = counts.max(axis=0)  # [G, NB, GT] static segment sizes

    # ---- static layout ----
    # per (g,b): num_idxs, chunk count, idx col count; offsets
    nidx = seg.sum(axis=2)  # [G, NB]
    ct = (nidx + P - 1) // P
    icb = (nidx + 15) // 16
    co_in_g = np.cumsum(ct, axis=1) - ct  # chunk offset of (g,b) within group
    C_g = ct.sum(axis=1)  # chunks per group
    icols_g = icb.sum(axis=1)
    icol_off_g = np.concatenate([[0], np.cumsum(icols_g)])[:-1]
    icol_off_gb = icol_off_g[:, None] + (np.cumsum(icb, axis=1) - icb)
    total_icols = int(icols_g.sum())
    seg_off = np.cumsum(seg, axis=2) - seg  # slot offset of tile seg in call

    # matmul op program: per group, ordered by (bucket, chunk, tile)
    # op = (chunk_in_group, tile_in_group, s_col_engine, s_col_idx, start, stop)
    ops_per_group = []
    runs_per_group = []
    scol_dve = 0
    scol_act = 0
    run_idx = 0
    for g in range(N_GROUPS):
        ops = []
        for b in range(NB):
            for t, tn in enumerate(gtiles[g]):
                lo = int(seg_off[g, b, t])
                hi = lo + int(seg[g, b, t])
                if hi == lo:
                    continue
                c0, c1 = lo // P, (hi - 1) // P
                for c in range(c0, c1 + 1):
                    r0 = max(lo, c * P) - c * P
                    r1 = min(hi, (c + 1) * P) - c * P
                    ops.append(
                        dict(
                            chunk=int(co_in_g[g, b]) + c,
                            tin=t,
                            r0=r0,
                            r1=r1,
                            b=b,
                        )
                    )
        # start/stop flags per tile within group
        seen = {}
        for o in ops:
            if o["tin"] not in seen:
                o["start"] = True
                seen[o["tin"]] = o
            else:
                o["start"] = False
            o["stop"] = False
        last = {}
        for o in ops:
            last[o["tin"]] = o
        for o in last.values():
            o["stop"] = True
        ops_per_group.append(ops)
        # chop into runs; assign engines and s columns
        runs = []
        for i in range(0, len(ops), RUN_K):
            run = ops[i : i + RUN_K]
            use_dve = run_idx % ACT_EVERY != ACT_EVERY - 1
            if use_dve:
                sc0 = scol_dve
                for j, o in enumerate(run):
                    o["dve"] = True
                    o["scol"] = sc0 + j
                scol_dve += len(run)
            else:
                for o in run:
                    o["dve"] = False
                    o["scol"] = scol_act
                    scol_act += 1
                sc0 = None
            runs.append(dict(ops=run, dve=use_dve, scol0=sc0))
            run_idx += 1
        runs_per_group.append(runs)

    n_dve_cols = scol_dve
    n_act_cols = scol_act

    layout = dict(
        gtiles=gtiles,
        seg=seg,
        nidx=nidx,
        ct=ct,
        icb=icb,
        co_in_g=co_in_g,
        C_g=C_g,
        Cmax=int(C_g.max()),
        icol_off_gb=icol_off_gb,
        total_icols=total_icols,
        ops_per_group=ops_per_group,
        runs_per_group=runs_per_group,
        n_dve_cols=n_dve_cols,
        n_act_cols=n_act_cols,
    )

    # per-(g,b,t,chunk) -> op column lookup (static, shared by all cores)
    opcol = {}
    for g, ops in enumerate(ops_per_group):
        for o in ops:
            opcol[(g, o["chunk"])] = opcol.get((g, o["chunk"]), {})
            opcol[(g, o["chunk"])][o["tin"]] = (o["dve"], o["scol"])

    wt = np.ascontiguousarray(W.T).astype(bfloat16)  # wt[i, o] = W[o, i]
    iota_bf = np.tile(np.arange(P, dtype=np.float32), (P, 1)).astype(bfloat16)
    iota_f32 = np.tile(np.arange(P, dtype=np.float32), (P, 1))

    in_maps = []
    for k in range(N_CORES):
        src_s, bkt_s, gid, ld_s = per_core[k]
        # position of each edge within its (g,b,t) segment for this core
        cnt_flat = counts[k].reshape(-1)
        gstart = np.zeros(N_GROUPS * NB * GROUP_TILES, np.int64)
        gstart[1:] = np.cumsum(cnt_flat)[:-1]
        pos = np.arange(len(src_s)) - gstart[gid]
        g_of = gid // (NB * GROUP_TILES)
        b_of = (gid // GROUP_TILES) % NB
        t_of = gid % GROUP_TILES
        slot = seg_off[g_of, b_of, t_of] + pos  # slot within the (g,b) call

        idx16 = np.zeros((16, total_icols), np.int16)
        icol = icol_off_gb[g_of, b_of] + slot // 16
        idx16[slot % 16, icol] = (src_s - b_of * BUCKET_MAX).astype(np.int16)
        idxm = np.tile(idx16, (8, 1))

        ld_dve = np.full((P, max(n_dve_cols, 1)), -1.0, np.float32)
        ld_act = np.full((P, max(n_act_cols, 1)), -1.0, np.float32)
        chunk_in_g = co_in_g[g_of, b_of] + slot // P
        row = slot % P
        for j in range(len(src_s)):
            dve, sc = opcol[(g_of[j], chunk_in_g[j])][t_of[j]]
            if dve:
                ld_dve[row[j], sc] = ld_s[j]
            else:
                ld_act[row[j], sc] = ld_s[j]

        normd = np.ones((P, N_TILES), np.float32)
        base = k * NPC
        for t in range(N_TILES):
            rows = P if t < N_TILES - 1 else ROWS_LAST
            normd[:rows, t] = norm[base + t * P : base + t * P + rows]

        in_maps.append(
            {
                "feats": table,
                "idxm": np.ascontiguousarray(idxm),
                "ld_dve": np.ascontiguousarray(ld_dve.astype(bfloat16)),
                "ld_act": np.ascontiguousarray(ld_act),
                "wt": wt,
                "iota_bf": iota_bf,
                "iota_f32": iota_f32,
                "normd": normd,
                "resid": np.ascontiguousarray(features[base : base + NPC]),
            }
        )
    return in_maps, layout


def _build_program(layout):
    f32 = mybir.dt.float32
    bf16 = mybir.dt.bfloat16
    i16 = mybir.dt.int16
    gtiles = layout["gtiles"]
    nidx = layout["nidx"]
    ct = layout["ct"]
    icb = layout["icb"]
    co_in_g = layout["co_in_g"]
    C_g = layout["C_g"]
    Cmax = layout["Cmax"]
    icol_off_gb = layout["icol_off_gb"]
    ops_per_group = layout["ops_per_group"]
    n_dve = max(layout["n_dve_cols"], 1)
    n_act = max(layout["n_act_cols"], 1)

    nc = bacc.Bacc(num_swdge_queues=4)
    feats = nc.declare_dram_parameter("feats", [N_NODES, P], bf16, isOutput=False)
    idxm = nc.declare_dram_parameter(
        "idxm", [P, layout["total_icols"]], i16, isOutput=False
    )
    ld_dve_d = nc.declare_dram_parameter("ld_dve", [P, n_dve], bf16, isOutput=False)
    ld_act_d = nc.declare_dram_parameter("ld_act", [P, n_act], f32, isOutput=False)
    wt = nc.declare_dram_parameter("wt", [P, P], bf16, isOutput=False)
    iota_bf_d = nc.declare_dram_parameter("iota_bf", [P, P], bf16, isOutput=False)
    iota_f32_d = nc.declare_dram_parameter("iota_f32", [P, P], f32, isOutput=False)
    normd_d = nc.declare_dram_parameter("normd", [P, N_TILES], f32, isOutput=False)
    resid = nc.declare_dram_parameter("resid", [NPC, P], f32, isOutput=False)
    out = nc.declare_dram_parameter("out", [NPC, P], f32, isOutput=True)

    with TileContext(nc) as tc:
        with (
            tc.tile_pool(name="const", bufs=1) as constp,
            tc.tile_pool(name="meta", bufs=4) as metap,
            tc.tile_pool(name="x", bufs=3) as xp,
            tc.tile_pool(name="s", bufs=6) as sp,
            tc.tile_pool(name="zps", bufs=6, space="PSUM") as zpsp,
            tc.tile_pool(name="yps", bufs=2, space="PSUM") as ypsp,
            tc.tile_pool(name="post", bufs=4) as postp,
        ):
            wt_sb = constp.tile([P, P], bf16)
            nc.sync.dma_start(out=wt_sb[:], in_=wt[:, :])
            iota_b = constp.tile([P, P], bf16)
            nc.sync.dma_start(out=iota_b[:], in_=iota_bf_d[:, :])
            iota_f = constp.tile([P, P], f32)
            nc.sync.dma_start(out=iota_f[:], in_=iota_f32_d[:, :])
            # full ld tables stay resident (small)
            ld_dve_sb = constp.tile([P, n_dve], bf16)
            nc.sync.dma_start(out=ld_dve_sb[:], in_=ld_dve_d[:, :])
            ld_act_sb = constp.tile([P, n_act], f32)
            nc.sync.dma_start(out=ld_act_sb[:], in_=ld_act_d[:, :])
            normd_sb = constp.tile([P, N_TILES], f32)
            nc.sync.dma_start(out=normd_sb[:], in_=normd_d[:, :])

            for g in range(N_GROUPS):
                icols = int(icb[g].sum())
                mt_i = metap.tile([P, max(icols, 1)], i16, tag="mi")
                ic0 = int(icol_off_gb[g, 0])
                nc.sync.dma_start(out=mt_i[:, :icols], in_=idxm[:, ic0 : ic0 + icols])

                Cg = int(C_g[g])
                X_full = xp.tile([P, Cmax * P], bf16, tag="X")
                X = X_full[:, : Cg * P]
                for b in range(NB):
                    n_idx = int(nidx[g, b])
                    if n_idx == 0:
                        continue
                    co = int(co_in_g[g, b])
                    cb = int(ct[g, b])
                    io = int(icol_off_gb[g, b]) - ic0
                    icbb = int(icb[g, b])
                    if n_idx % P:
                        nc.scalar.memzero(X[:, (co + cb - 1) * P : (co + cb) * P])
                    nc.gpsimd.dma_gather(
                        out_ap=X[:, co * P : (co + cb) * P].rearrange(
                            "p (c e) -> p c e", e=P
                        ),
                        in_ap=feats[
                            b * BUCKET_MAX : min((b + 1) * BUCKET_MAX, N_NODES), :
                        ],
                        idxs_ap=mt_i[:, io : io + icbb],
                        num_idxs=n_idx,
                        num_idxs_reg=n_idx,
                        elem_size=P,
                        single_packet=False,
                        queue_num=b % 4,
                    )

                z_ps = {}
                for run in layout["runs_per_group"][g]:
                    ops = run["ops"]
                    for o in ops:
                        if o["start"]:
                            z_ps[o["tin"]] = zpsp.tile(
                                [P, P], f32, tag="z", name=f"z{g}_{o['tin']}"
                            )
                    if run["dve"]:
                        kk = len(ops)
                        sc0 = run["scol0"]
                        S_big = sp.tile([P, RUN_K * P], bf16, tag="S")
                        nc.vector.tensor_tensor(
                            out=S_big[:, : kk * P].rearrange("p (c e) -> p c e", e=P),
                            in0=iota_b[:, :].unsqueeze(1).to_broadcast([P, kk, P]),
                            in1=ld_dve_sb[:, sc0 : sc0 + kk]
                            .unsqueeze(2)
                            .to_broadcast([P, kk, P]),
                            op=mybir.AluOpType.is_equal,
                        )
                        for j, o in enumerate(ops):
                            c = o["chunk"]
                            nc.tensor.matmul(
                                out=z_ps[o["tin"]][:],
                                lhsT=X[:, c * P : (c + 1) * P],
                                rhs=S_big[:, j * P : (j + 1) * P],
                                start=o["start"],
                                stop=o["stop"],
                            )
                    else:
                        for o in ops:
                            S = sp.tile([P, P], bf16, tag="Sa")
                            t2 = sp.tile([P, P], f32, tag="T2")
                            nc.scalar.activation(
                                out=t2[:],
                                in_=iota_f[:],
                                func=mybir.ActivationFunctionType.Square,
                                bias=ld_act_sb[:, o["scol"] : o["scol"] + 1],
                                scale=-1.0,
                            )
                            nc.scalar.activation(
                                out=S[:],
                                in_=t2[:],
                                func=mybir.ActivationFunctionType.Relu,
                                bias=1.0,
                                scale=-1.0,
                            )
                            c = o["chunk"]
                            nc.tensor.matmul(
                                out=z_ps[o["tin"]][:],
                                lhsT=X[:, c * P : (c + 1) * P],
                                rhs=S[:],
                                start=o["start"],
                                stop=o["stop"],
                            )

                for tin, tn in enumerate(gtiles[g]):
                    zT_sb = postp.tile([P, P], bf16, tag="zT")
                    nc.scalar.copy(out=zT_sb[:], in_=z_ps[tin][:])
                    y_ps = ypsp.tile([P, P], f32)
                    nc.tensor.matmul(
                        out=y_ps[:], lhsT=zT_sb[:], rhs=wt_sb[:], start=True, stop=True
                    )
                    rows = P if tn < N_TILES - 1 else ROWS_LAST
                    y_sb = postp.tile([P, P], f32, tag="y")
                    nc.scalar.activation(
                        out=y_sb[:],
                        in_=y_ps[:],
                        func=mybir.ActivationFunctionType.Relu,
                        scale=normd_sb[:, tn : tn + 1],
                    )
                    res_sb = postp.tile([P, P], f32, tag="res")
                    nc.sync.dma_start(
                        out=res_sb[:rows], in_=resid[tn * P : tn * P + rows, :]
                    )
                    o_sb = postp.tile([P, P], f32, tag="o")
                    nc.vector.tensor_add(
                        out=o_sb[:rows], in0=y_sb[:rows], in1=res_sb[:rows]
                    )
                    nc.sync.dma_start(
                        out=out[tn * P : tn * P + rows, :], in_=o_sb[:rows]
                    )
    nc.finalize()
    return nc


def _run(features, W, edge_src, edge_dst, trace=False, **spmd_kwargs):
    in_maps, layout = _prepare(features, W, edge_src, edge_dst)
    nc = _build_program(layout)
    br = run_bass_kernel_spmd(
        nc, in_maps, core_ids=list(range(N_CORES)), trace=trace, **spmd_kwargs
    )
    outs = [r["out"] for r in br.results]
    full = np.concatenate(outs, axis=0).astype(np.float32)
    return full, br


def kernel(features, W, edge_src, edge_dst):
    out, _ = _run(features, W, edge_src, edge_dst, trace=False)
    return out


# revision 12
# speedup vs baseline: 1.0706x; 1.0706x over previous
"""GCN layer (message passing) on 8 Trainium2 NeuronCores.

out = relu(((D^-1/2 A D^-1/2) X) @ W.T) + X

v2 strategy (bf16 datapath, dst-sharded):
  - Destination nodes sharded across 8 cores (12500 each). Host bakes
    norm[src] into a bf16 feature table (so the one-hot S is binary) and
    applies norm[dst] as the per-partition scale of the final ReLU.
  - Edges sorted by (tile-group of 4 dst tiles, src bucket, dst tile, src).
    One dma_gather per (group, bucket) -> ~100 calls/core instead of 392,
    cutting the serial Q7 SWDGE descriptor-generation time.
  - Per 128-edge chunk: S[e, d] = (ld_e == d) built on DVE (tensor_scalar
    is_equal, bf16) or ACT (two activations, fp32->bf16), alternating to
    balance engines; PE accumulates zT[i, d] += X_c[e, i].T @ S_c[e, d]
    in fp32 PSUM from bf16 operands (no fp32 LOW/HIGH matmul split).
  - Per dst tile: zT -> bf16 SBUF copy, y = zT.T @ W.T on PE,
    relu(norm_dst * y) on ACT (per-partition scale), +residual on DVE.
  - SPMD static sizes: per (group, bucket, tile) segment = max count over
    the 8 cores; pad slots gather row 0 of the bucket and carry ld = -1
    (zero one-hot row). Unwritten tail slots of the last chunk of each
    gather call are memzeroed so 0 * garbage cannot poison PSUM.
"""

import math

import numpy as np
from ml_dtypes import bfloat16

import concourse.bacc as bacc
import concourse.mybir as mybir
from concourse.bass_utils import run_bass_kernel_spmd
from concourse.tile import TileContext

P = 128
N_CORES = 8
BUCKET_MAX = 25000  # int16 gather indices: bucket the node space
GROUP_TILES = 4  # dst tiles per gather group

N_NODES = 100000
NPC = N_NODES // N_CORES  # 12500
N_TILES = math.ceil(NPC / P)  # 98
ROWS_LAST = NPC - (N_TILES - 1) * P  # 84
NB = math.ceil(N_NODES / BUCKET_MAX)  # 4
N_GROUPS = math.ceil(N_TILES / GROUP_TILES)  # 25

RUN_K = 8  # chunks per grouped DVE S-build
ACT_EVERY = 10  # every ACT_EVERY-th run goes to the ACT 2-op path


def _prepare(features, W, edge_src, edge_dst):
    features = np.asarray(features, dtype=np.float32)
    W = np.asarray(W, dtype=np.float32)
    edge_src = np.asarray(edge_src, dtype=np.int32)
    edge_dst = np.asarray(edge_dst, dtype=np.int32)

    n_nodes, d = features.shape
    assert d == P and n_nodes == N_NODES

    degs = np.bincount(edge_dst, minlength=n_nodes).astype(np.float32)
    norm = 1.0 / np.sqrt(np.maximum(degs, 1.0), dtype=np.float32)
    table = (features * norm[:, None]).astype(bfloat16)  # norm[src] baked in

    core_of = edge_dst // NPC

    # tiles per group (last group may be short)
    gtiles = [
        list(range(g * GROUP_TILES, min((g + 1) * GROUP_TILES, N_TILES)))
        for g in range(N_GROUPS)
    ]

    # per-core sorted edges and per-(group,bucket,tile) counts
    per_core = []
    counts = np.zeros((N_CORES, N_GROUPS, NB, GROUP_TILES), np.int64)
    for k in range(N_CORES):
        sel = np.flatnonzero(core_of == k)
        src_k = edge_src[sel]
        ldst = edge_dst[sel] - k * NPC
        tile = ldst >> 7
        grp = tile // GROUP_TILES
        tin = tile % GROUP_TILES  # tile index within group
        bkt = src_k // BUCKET_MAX
        order = np.lexsort((src_k, tin, bkt, grp))
        src_s = src_k[order]
        grp_s, bkt_s, tin_s = grp[order], bkt[order], tin[order]
        ld_s = (ldst[order] & 127).astype(np.float32)
        gid = (grp_s * NB + bkt_s) * GROUP_TILES + tin_s
        cnt = np.bincount(gid, minlength=N_GROUPS * NB * GROUP_TILES)
        counts[k] = cnt.reshape(N_GROUPS, NB, GROUP_TILES)
        per_core.append((src_s, bkt_s, gid, ld_s))

    seg = counts.max(axis=0)  # [G, NB, GT] static segment sizes

    # ---- static layout ----
    # per (g,b): num_idxs, chunk count, idx col count; offsets
    nidx = seg.sum(axis=2)  # [G, NB]
    ct = (nidx + P - 1) // P
    icb = (nidx + 15) // 16
    co_in_g = np.cumsum(ct, axis=1) - ct  # chunk offset of (g,b) within group
    C_g = ct.sum(axis=1)  # chunks per group
    icols_g = icb.sum(axis=1)
    icol_off_g = np.concatenate([[0], np.cumsum(icols_g)])[:-1]
    icol_off_gb = icol_off_g[:, None] + (np.cumsum(icb, axis=1) - icb)
    total_icols = int(icols_g.sum())
    seg_off = np.cumsum(seg, axis=2) - seg  # slot offset of tile seg in call

    # matmul op program: per group, ordered by (bucket, chunk, tile)
    # op = (chunk_in_group, tile_in_group, s_col_engine, s_col_idx, start, stop)
    ops_per_group = []
    runs_per_group = []
    scol_dve = 0
    scol_act = 0
    run_idx = 0
    for g in range(N_GROUPS):
        ops = []
        for b in range(NB):
            for t, tn in enumerate(gtiles[g]):
                lo = int(seg_off[g, b, t])
                hi = lo + int(seg[g, b, t])
                if hi == lo:
                    continue
                c0, c1 = lo // P, (hi - 1) // P
                for c in range(c0, c1 + 1):
                    r0 = max(lo, c * P) - c * P
                    r1 = min(hi, (c + 1) * P) - c * P
                    ops.append(
                        dict(
                            chunk=int(co_in_g[g, b]) + c,
                            tin=t,
                            r0=r0,
                            r1=r1,
                            b=b,
                        )
                    )
        # start/stop flags per tile within group
        seen = {}
        for o in ops:
            if o["tin"] not in seen:
                o["start"] = True
                seen[o["tin"]] = o
            else:
                o["start"] = False
            o["stop"] = False
        last = {}
        for o in ops:
            last[o["tin"]] = o
        for o in last.values():
            o["stop"] = True
        ops_per_group.append(ops)
        # chop into runs; assign engines and s columns
        runs = []
        for i in range(0, len(ops), RUN_K):
            run = ops[i : i + RUN_K]
            use_dve = run_idx % ACT_EVERY != ACT_EVERY - 1
            if use_dve:
                sc0 = scol_dve
                for j, o in enumerate(run):
                    o["dve"] = True
                    o["scol"] = sc0 + j
                scol_dve += len(run)
            else:
                for o in run:
                    o["dve"] = False
                    o["scol"] = scol_act
                    scol_act += 1
                sc0 = None
            runs.append(dict(ops=run, dve=use_dve, scol0=sc0))
            run_idx += 1
        runs_per_group.append(runs)

    n_dve_cols = scol_dve
    n_act_cols = scol_act

    layout = dict(
        gtiles=gtiles,
        seg=seg,
        nidx=nidx,
        ct=ct,
        icb=icb,
        co_in_g=co_in_g,
        C_g=C_g,
        Cmax=int(C_g.max()),
        icol_off_gb=icol_off_gb,
        total_icols=total_icols,
        ops_per_group=ops_per_group,
        runs_per_group=runs_per_group,
        n_dve_cols=n_dve_cols,
        n_act_cols=n_act_cols,
    )

    # per-(g,b,t,chunk) -> op column lookup (static, shared by all cores)
    opcol = {}
    for g, ops in enumerate(ops_per_group):
        for o in ops:
            opcol[(g, o["chunk"])] = opcol.get((g, o["chunk"]), {})
            opcol[(g, o["chunk"])][o["tin"]] = (o["dve"], o["scol"])

    wt = np.ascontiguousarray(W.T).astype(bfloat16)  # wt[i, o] = W[o, i]
    iota_bf = np.tile(np.arange(P, dtype=np.float32), (P, 1)).astype(bfloat16)
    iota_f32 = np.tile(np.arange(P, dtype=np.float32), (P, 1))

    in_maps = []
    for k in range(N_CORES):
        src_s, bkt_s, gid, ld_s = per_core[k]
        # position of each edge within its (g,b,t) segment for this core
        cnt_flat = counts[k].reshape(-1)
        gstart = np.zeros(N_GROUPS * NB * GROUP_TILES, np.int64)
        gstart[1:] = np.cumsum(cnt_flat)[:-1]
        pos = np.arange(len(src_s)) - gstart[gid]
        g_of = gid // (NB * GROUP_TILES)
        b_of = (gid // GROUP_TILES) % NB
        t_of = gid % GROUP_TILES
        slot = seg_off[g_of, b_of, t_of] + pos  # slot within the (g,b) call

        idx16 = np.zeros((16, total_icols), np.int16)
        icol = icol_off_gb[g_of, b_of] + slot // 16
        idx16[slot % 16, icol] = (src_s - b_of * BUCKET_MAX).astype(np.int16)
        idxm = np.tile(idx16, (8, 1))

        ld_dve = np.full((P, max(n_dve_cols, 1)), -1.0, np.float32)
        ld_act = np.full((P, max(n_act_cols, 1)), -1.0, np.float32)
        chunk_in_g = co_in_g[g_of, b_of] + slot // P
        row = slot % P
        for j in range(len(src_s)):
            dve, sc = opcol[(g_of[j], chunk_in_g[j])][t_of[j]]
            if dve:
                ld_dve[row[j], sc] = ld_s[j]
            else:
                ld_act[row[j], sc] = ld_s[j]

        normd = np.ones((P, N_TILES), np.float32)
        base = k * NPC
        for t in range(N_TILES):
            rows = P if t < N_TILES - 1 else ROWS_LAST
            normd[:rows, t] = norm[base + t * P : base + t * P + rows]

        in_maps.append(
            {
                "feats": table,
                "idxm": np.ascontiguousarray(idxm),
                "ld_dve": np.ascontiguousarray(ld_dve.astype(bfloat16)),
                "ld_act": np.ascontiguousarray(ld_act),
                "wt": wt,
                "iota_bf": iota_bf,
                "iota_f32": iota_f32,
                "normd": normd,
                "resid": np.ascontiguousarray(features[base : base + NPC]),
            }
        )
    return in_maps, layout


def _build_program(layout):
    f32 = mybir.dt.float32
    bf16 = mybir.dt.bfloat16
    i16 = mybir.dt.int16
    gtiles = layout["gtiles"]
    nidx = layout["nidx"]
    ct = layout["ct"]
    icb = layout["icb"]
    co_in_g = layout["co_in_g"]
    C_g = layout["C_g"]
    Cmax = layout["Cmax"]
    icol_off_gb = layout["icol_off_gb"]
    ops_per_group = layout["ops_per_group"]
    n_dve = max(layout["n_dve_cols"], 1)
    n_act = max(layout["n_act_cols"], 1)

    nc = bacc.Bacc(num_swdge_queues=4, dynamic_dma_scratch_size=32768)
    feats = nc.declare_dram_parameter("feats", [N_NODES, P], bf16, isOutput=False)
    idxm = nc.declare_dram_parameter(
        "idxm", [P, layout["total_icols"]], i16, isOutput=False
    )
    ld_dve_d = nc.declare_dram_parameter("ld_dve", [P, n_dve], bf16, isOutput=False)
    ld_act_d = nc.declare_dram_parameter("ld_act", [P, n_act], f32, isOutput=False)
    wt = nc.declare_dram_parameter("wt", [P, P], bf16, isOutput=False)
    iota_bf_d = nc.declare_dram_parameter("iota_bf", [P, P], bf16, isOutput=False)
    iota_f32_d = nc.declare_dram_parameter("iota_f32", [P, P], f32, isOutput=False)
    normd_d = nc.declare_dram_parameter("normd", [P, N_TILES], f32, isOutput=False)
    resid = nc.declare_dram_parameter("resid", [NPC, P], f32, isOutput=False)
    out = nc.declare_dram_parameter("out", [NPC, P], f32, isOutput=True)

    with TileContext(nc) as tc:
        with (
            tc.tile_pool(name="const", bufs=1) as constp,
            tc.tile_pool(name="meta", bufs=4) as metap,
            tc.tile_pool(name="x", bufs=3) as xp,
            tc.tile_pool(name="s", bufs=6) as sp,
            tc.tile_pool(name="zps", bufs=6, space="PSUM") as zpsp,
            tc.tile_pool(name="yps", bufs=2, space="PSUM") as ypsp,
            tc.tile_pool(name="post", bufs=4) as postp,
        ):
            wt_sb = constp.tile([P, P], bf16)
            nc.sync.dma_start(out=wt_sb[:], in_=wt[:, :])
            iota_b = constp.tile([P, P], bf16)
            nc.sync.dma_start(out=iota_b[:], in_=iota_bf_d[:, :])
            iota_f = constp.tile([P, P], f32)
            nc.sync.dma_start(out=iota_f[:], in_=iota_f32_d[:, :])
            # full ld tables stay resident (small)
            ld_dve_sb = constp.tile([P, n_dve], bf16)
            nc.sync.dma_start(out=ld_dve_sb[:], in_=ld_dve_d[:, :])
            ld_act_sb = constp.tile([P, n_act], f32)
            nc.sync.dma_start(out=ld_act_sb[:], in_=ld_act_d[:, :])
            normd_sb = constp.tile([P, N_TILES], f32)
            nc.sync.dma_start(out=normd_sb[:], in_=normd_d[:, :])

            for g in range(N_GROUPS):
                icols = int(icb[g].sum())
                mt_i = metap.tile([P, max(icols, 1)], i16, tag="mi")
                ic0 = int(icol_off_gb[g, 0])
                nc.sync.dma_start(out=mt_i[:, :icols], in_=idxm[:, ic0 : ic0 + icols])

                Cg = int(C_g[g])
                X_full = xp.tile([P, Cmax * P], bf16, tag="X")
                X = X_full[:, : Cg * P]
                for b in range(NB):
                    n_idx = int(nidx[g, b])
                    if n_idx == 0:
                        continue
                    co = int(co_in_g[g, b])
                    cb = int(ct[g, b])
                    io = int(icol_off_gb[g, b]) - ic0
                    icbb = int(icb[g, b])
                    if n_idx % P:
                        nc.scalar.memzero(X[:, (co + cb - 1) * P : (co + cb) * P])
                    nc.gpsimd.dma_gather(
                        out_ap=X[:, co * P : (co + cb) * P].rearrange(
                            "p (c e) -> p c e", e=P
                        ),
                        in_ap=feats[
                            b * BUCKET_MAX : min((b + 1) * BUCKET_MAX, N_NODES), :
                        ],
                        idxs_ap=mt_i[:, io : io + icbb],
                        num_idxs=n_idx,
                        num_idxs_reg=n_idx,
                        elem_size=P,
                        single_packet=False,
                        queue_num=b % 4,
                    )

                z_ps = {}
                for run in layout["runs_per_group"][g]:
                    ops = run["ops"]
                    for o in ops:
                        if o["start"]:
                            z_ps[o["tin"]] = zpsp.tile(
                                [P, P], f32, tag="z", name=f"z{g}_{o['tin']}"
                            )
                    if run["dve"]:
                        kk = len(ops)
                        sc0 = run["scol0"]
                        S_big = sp.tile([P, RUN_K * P], bf16, tag="S")
                        nc.vector.tensor_tensor(
                            out=S_big[:, : kk * P].rearrange("p (c e) -> p c e", e=P),
                            in0=iota_b[:, :].unsqueeze(1).to_broadcast([P, kk, P]),
                            in1=ld_dve_sb[:, sc0 : sc0 + kk]
                            .unsqueeze(2)
                            .to_broadcast([P, kk, P]),
                            op=mybir.AluOpType.is_equal,
                        )
                        for j, o in enumerate(ops):
                            c = o["chunk"]
                            nc.tensor.matmul(
                                out=z_ps[o["tin"]][:],
                                lhsT=X[:, c * P : (c + 1) * P],
                                rhs=S_big[:, j * P : (j + 1) * P],
                                start=o["start"],
                                stop=o["stop"],
                            )
                    else:
                        for o in ops:
                            S = sp.tile([P, P], bf16, tag="Sa")
                            t2 = sp.tile([P, P], f32, tag="T2")
                            nc.scalar.activation(
                                out=t2[:],
                                in_=iota_f[:],
                                func=mybir.ActivationFunctionType.Square,
                                bias=ld_act_sb[:, o["scol"] : o["scol"] + 1],
                                scale=-1.0,
                            )
                            nc.scalar.activation(
                                out=S[:],
                                in_=t2[:],
                                func=mybir.ActivationFunctionType.Relu,
                                bias=1.0,
                                scale=-1.0,
                            )
                            c = o["chunk"]
                            nc.tensor.matmul(
                                out=z_ps[o["tin"]][:],
                                lhsT=X[:, c * P : (c + 1) * P],
                                rhs=S[:],
                                start=o["start"],
                                stop=o["stop"],
                            )

                for tin, tn in enumerate(gtiles[g]):
                    zT_sb = postp.tile([P, P], bf16, tag="zT")
                    nc.scalar.copy(out=zT_sb[:], in_=z_ps[tin][:])
                    y_ps = ypsp.tile([P, P], f32)
                    nc.tensor.matmul(
                        out=y_ps[:], lhsT=zT_sb[:], rhs=wt_sb[:], start=True, stop=True
                    )
                    rows = P if tn < N_TILES - 1 else ROWS_LAST
                    y_sb = postp.tile([P, P], f32, tag="y")
                    nc.scalar.activation(
                        out=y_sb[:],
                        in_=y_ps[:],
                        func=mybir.ActivationFunctionType.Relu,
                        scale=normd_sb[:, tn : tn + 1],
                    )
                    res_sb = postp.tile([P, P], f32, tag="res")
                    nc.sync.dma_start(
                        out=res_sb[:rows], in_=resid[tn * P : tn * P + rows, :]
                    )
                    o_sb = postp.tile([P, P], f32, tag="o")
                    nc.vector.tensor_add(
                        out=o_sb[:rows], in0=y_sb[:rows], in1=res_sb[:rows]
                    )
                    nc.sync.dma_start(
                        out=out[tn * P : tn * P + rows, :], in_=o_sb[:rows]
                    )
    nc.finalize()
    return nc


def _run(features, W, edge_src, edge_dst, trace=False, **spmd_kwargs):
    in_maps, layout = _prepare(features, W, edge_src, edge_dst)
    nc = _build_program(layout)
    br = run_bass_kernel_spmd(
        nc, in_maps, core_ids=list(range(N_CORES)), trace=trace, **spmd_kwargs
    )
    outs = [r["out"] for r in br.results]
    full = np.concatenate(outs, axis=0).astype(np.float32)
    return full, br


def kernel(features, W, edge_src, edge_dst):
    out, _ = _run(features, W, edge_src, edge_dst, trace=False)
    return out


# revision 13
# speedup vs baseline: 1.1090x; 1.0358x over previous
"""GCN layer (message passing) on 8 Trainium2 NeuronCores.

out = relu(((D^-1/2 A D^-1/2) X) @ W.T) + X

v2 strategy (bf16 datapath, dst-sharded):
  - Destination nodes sharded across 8 cores (12500 each). Host bakes
    norm[src] into a bf16 feature table (so the one-hot S is binary) and
    applies norm[dst] as the per-partition scale of the final ReLU.
  - Edges sorted by (tile-group of 4 dst tiles, src bucket, dst tile, src).
    One dma_gather per (group, bucket) -> ~100 calls/core instead of 392,
    cutting the serial Q7 SWDGE descriptor-generation time.
  - Per 128-edge chunk: S[e, d] = (ld_e == d) built on DVE (tensor_scalar
    is_equal, bf16) or ACT (two activations, fp32->bf16), alternating to
    balance engines; PE accumulates zT[i, d] += X_c[e, i].T @ S_c[e, d]
    in fp32 PSUM from bf16 operands (no fp32 LOW/HIGH matmul split).
  - Per dst tile: zT -> bf16 SBUF copy, y = zT.T @ W.T on PE,
    relu(norm_dst * y) on ACT (per-partition scale), +residual on DVE.
  - SPMD static sizes: per (group, bucket, tile) segment = max count over
    the 8 cores; pad slots gather row 0 of the bucket and carry ld = -1
    (zero one-hot row). Unwritten tail slots of the last chunk of each
    gather call are memzeroed so 0 * garbage cannot poison PSUM.
"""

import math

import numpy as np
from ml_dtypes import bfloat16

import concourse.bacc as bacc
import concourse.mybir as mybir
from concourse.bass_utils import run_bass_kernel_spmd
from concourse.tile import TileContext

P = 128
N_CORES = 8
BUCKET_MAX = 25000  # int16 gather indices: bucket the node space
GROUP_TILES = 4  # dst tiles per gather group

N_NODES = 100000
NPC = N_NODES // N_CORES  # 12500
N_TILES = math.ceil(NPC / P)  # 98
ROWS_LAST = NPC - (N_TILES - 1) * P  # 84
NB = math.ceil(N_NODES / BUCKET_MAX)  # 4
N_GROUPS = math.ceil(N_TILES / GROUP_TILES)  # 25

RUN_K = 8  # chunks per grouped DVE S-build
ACT_EVERY = 10  # every ACT_EVERY-th run goes to the ACT 2-op path


def _prepare(features, W, edge_src, edge_dst):
    features = np.asarray(features, dtype=np.float32)
    W = np.asarray(W, dtype=np.float32)
    edge_src = np.asarray(edge_src, dtype=np.int32)
    edge_dst = np.asarray(edge_dst, dtype=np.int32)

    n_nodes, d = features.shape
    assert d == P and n_nodes == N_NODES

    degs = np.bincount(edge_dst, minlength=n_nodes).astype(np.float32)
    norm = 1.0 / np.sqrt(np.maximum(degs, 1.0), dtype=np.float32)
    table = (features * norm[:, None]).astype(bfloat16)  # norm[src] baked in

    core_of = edge_dst // NPC

    # tiles per group (last group may be short)
    gtiles = [
        list(range(g * GROUP_TILES, min((g + 1) * GROUP_TILES, N_TILES)))
        for g in range(N_GROUPS)
    ]

    # per-core sorted edges and per-(group,bucket,tile) counts
    per_core = []
    counts = np.zeros((N_CORES, N_GROUPS, NB, GROUP_TILES), np.int64)
    for k in range(N_CORES):
        sel = np.flatnonzero(core_of == k)
        src_k = edge_src[sel]
        ldst = edge_dst[sel] - k * NPC
        tile = ldst >> 7
        grp = tile // GROUP_TILES
        tin = tile % GROUP_TILES  # tile index within group
        bkt = src_k // BUCKET_MAX
        order = np.lexsort((src_k, tin, bkt, grp))
        src_s = src_k[order]
        grp_s, bkt_s, tin_s = grp[order], bkt[order], tin[order]
        ld_s = (ldst[order] & 127).astype(np.float32)
        gid = (grp_s * NB + bkt_s) * GROUP_TILES + tin_s
        cnt = np.bincount(gid, minlength=N_GROUPS * NB * GROUP_TILES)
        counts[k] = cnt.reshape(N_GROUPS, NB, GROUP_TILES)
        per_core.append((src_s, bkt_s, gid, ld_s))

    seg = counts.max(axis=0)  # [G, NB, GT] static segment sizes

    # ---- static layout ----
    # per (g,b): num_idxs, chunk count, idx col count; offsets
    nidx = seg.sum(axis=2)  # [G, NB]
    ct = (nidx + P - 1) // P
    icb = (nidx + 15) // 16
    co_in_g = np.cumsum(ct, axis=1) - ct  # chunk offset of (g,b) within group
    C_g = ct.sum(axis=1)  # chunks per group
    icols_g = icb.sum(axis=1)
    icol_off_g = np.concatenate([[0], np.cumsum(icols_g)])[:-1]
    icol_off_gb = icol_off_g[:, None] + (np.cumsum(icb, axis=1) - icb)
    total_icols = int(icols_g.sum())
    seg_off = np.cumsum(seg, axis=2) - seg  # slot offset of tile seg in call

    # matmul op program: per group, ordered by (bucket, chunk, tile)
    # op = (chunk_in_group, tile_in_group, s_col_engine, s_col_idx, start, stop)
    ops_per_group = []
    runs_per_group = []
    scol_dve = 0
    scol_act = 0
    run_idx = 0
    for g in range(N_GROUPS):
        ops = []
        for b in range(NB):
            for t, tn in enumerate(gtiles[g]):
                lo = int(seg_off[g, b, t])
                hi = lo + int(seg[g, b, t])
                if hi == lo:
                    continue
                c0, c1 = lo // P, (hi - 1) // P
                for c in range(c0, c1 + 1):
                    r0 = max(lo, c * P) - c * P
                    r1 = min(hi, (c + 1) * P) - c * P
                    ops.append(
                        dict(
                            chunk=int(co_in_g[g, b]) + c,
                            tin=t,
                            r0=r0,
                            r1=r1,
                            b=b,
                        )
                    )
        # start/stop flags per tile within group
        seen = {}
        for o in ops:
            if o["tin"] not in seen:
                o["start"] = True
                seen[o["tin"]] = o
            else:
                o["start"] = False
            o["stop"] = False
        last = {}
        for o in ops:
            last[o["tin"]] = o
        for o in last.values():
            o["stop"] = True
        ops_per_group.append(ops)
        # chop into runs; assign engines and s columns
        runs = []
        for i in range(0, len(ops), RUN_K):
            run = ops[i : i + RUN_K]
            use_dve = run_idx % ACT_EVERY != ACT_EVERY - 1
            if use_dve:
                sc0 = scol_dve
                for j, o in enumerate(run):
                    o["dve"] = True
                    o["scol"] = sc0 + j
                scol_dve += len(run)
            else:
                for o in run:
                    o["dve"] = False
                    o["scol"] = scol_act
                    scol_act += 1
                sc0 = None
            runs.append(dict(ops=run, dve=use_dve, scol0=sc0))
            run_idx += 1
        runs_per_group.append(runs)

    n_dve_cols = scol_dve
    n_act_cols = scol_act

    layout = dict(
        gtiles=gtiles,
        seg=seg,
        nidx=nidx,
        ct=ct,
        icb=icb,
        co_in_g=co_in_g,
        C_g=C_g,
        Cmax=int(C_g.max()),
        icol_off_gb=icol_off_gb,
        total_icols=total_icols,
        ops_per_group=ops_per_group,
        runs_per_group=runs_per_group,
        n_dve_cols=n_dve_cols,
        n_act_cols=n_act_cols,
    )

    # per-(g,b,t,chunk) -> op column lookup (static, shared by all cores)
    opcol = {}
    for g, ops in enumerate(ops_per_group):
        for o in ops:
            opcol[(g, o["chunk"])] = opcol.get((g, o["chunk"]), {})
            opcol[(g, o["chunk"])][o["tin"]] = (o["dve"], o["scol"])

    wt = np.ascontiguousarray(W.T).astype(bfloat16)  # wt[i, o] = W[o, i]
    iota_bf = np.tile(np.arange(P, dtype=np.float32), (P, 1)).astype(bfloat16)
    iota_f32 = np.tile(np.arange(P, dtype=np.float32), (P, 1))

    in_maps = []
    for k in range(N_CORES):
        src_s, bkt_s, gid, ld_s = per_core[k]
        # position of each edge within its (g,b,t) segment for this core
        cnt_flat = counts[k].reshape(-1)
        gstart = np.zeros(N_GROUPS * NB * GROUP_TILES, np.int64)
        gstart[1:] = np.cumsum(cnt_flat)[:-1]
        pos = np.arange(len(src_s)) - gstart[gid]
        g_of = gid // (NB * GROUP_TILES)
        b_of = (gid // GROUP_TILES) % NB
        t_of = gid % GROUP_TILES
        slot = seg_off[g_of, b_of, t_of] + pos  # slot within the (g,b) call

        idx16 = np.zeros((16, total_icols), np.int16)
        icol = icol_off_gb[g_of, b_of] + slot // 16
        idx16[slot % 16, icol] = (src_s - b_of * BUCKET_MAX).astype(np.int16)
        idxm = np.tile(idx16, (8, 1))

        ld_dve = np.full((P, max(n_dve_cols, 1)), -1.0, np.float32)
        ld_act = np.full((P, max(n_act_cols, 1)), -1.0, np.float32)
        chunk_in_g = co_in_g[g_of, b_of] + slot // P
        row = slot % P
        for j in range(len(src_s)):
            dve, sc = opcol[(g_of[j], chunk_in_g[j])][t_of[j]]
            if dve:
                ld_dve[row[j], sc] = ld_s[j]
            else:
                ld_act[row[j], sc] = ld_s[j]

        normd = np.ones((P, N_TILES), np.float32)
        base = k * NPC
        for t in range(N_TILES):
            rows = P if t < N_TILES - 1 else ROWS_LAST
            normd[:rows, t] = norm[base + t * P : base + t * P + rows]

        in_maps.append(
            {
                "feats": table,
                "idxm": np.ascontiguousarray(idxm),
                "ld_dve": np.ascontiguousarray(ld_dve.astype(bfloat16)),
                "ld_act": np.ascontiguousarray(ld_act),
                "wt": wt,
                "iota_bf": iota_bf,
                "iota_f32": iota_f32,
                "normd": normd,
                "resid": np.ascontiguousarray(features[base : base + NPC]),
            }
        )
    return in_maps, layout


def _build_program(layout):
    f32 = mybir.dt.float32
    bf16 = mybir.dt.bfloat16
    i16 = mybir.dt.int16
    gtiles = layout["gtiles"]
    nidx = layout["nidx"]
    ct = layout["ct"]
    icb = layout["icb"]
    co_in_g = layout["co_in_g"]
    C_g = layout["C_g"]
    Cmax = layout["Cmax"]
    icol_off_gb = layout["icol_off_gb"]
    ops_per_group = layout["ops_per_group"]
    n_dve = max(layout["n_dve_cols"], 1)
    n_act = max(layout["n_act_cols"], 1)

    nc = bacc.Bacc(num_swdge_queues=4)
    feats = nc.declare_dram_parameter("feats", [N_NODES, P], bf16, isOutput=False)
    idxm = nc.declare_dram_parameter(
        "idxm", [P, layout["total_icols"]], i16, isOutput=False
    )
    ld_dve_d = nc.declare_dram_parameter("ld_dve", [P, n_dve], bf16, isOutput=False)
    ld_act_d = nc.declare_dram_parameter("ld_act", [P, n_act], f32, isOutput=False)
    wt = nc.declare_dram_parameter("wt", [P, P], bf16, isOutput=False)
    iota_bf_d = nc.declare_dram_parameter("iota_bf", [P, P], bf16, isOutput=False)
    iota_f32_d = nc.declare_dram_parameter("iota_f32", [P, P], f32, isOutput=False)
    normd_d = nc.declare_dram_parameter("normd", [P, N_TILES], f32, isOutput=False)
    resid = nc.declare_dram_parameter("resid", [NPC, P], f32, isOutput=False)
    out = nc.declare_dram_parameter("out", [NPC, P], f32, isOutput=True)

    with TileContext(nc) as tc:
        with (
            tc.tile_pool(name="const", bufs=1) as constp,
            tc.tile_pool(name="meta", bufs=4) as metap,
            tc.tile_pool(name="x", bufs=4) as xp,
            tc.tile_pool(name="s", bufs=6) as sp,
            tc.tile_pool(name="zps", bufs=6, space="PSUM") as zpsp,
            tc.tile_pool(name="yps", bufs=2, space="PSUM") as ypsp,
            tc.tile_pool(name="post", bufs=4) as postp,
        ):
            wt_sb = constp.tile([P, P], bf16)
            nc.sync.dma_start(out=wt_sb[:], in_=wt[:, :])
            iota_b = constp.tile([P, P], bf16)
            nc.sync.dma_start(out=iota_b[:], in_=iota_bf_d[:, :])
            iota_f = constp.tile([P, P], f32)
            nc.sync.dma_start(out=iota_f[:], in_=iota_f32_d[:, :])
            # full ld tables stay resident (small)
            ld_dve_sb = constp.tile([P, n_dve], bf16)
            nc.sync.dma_start(out=ld_dve_sb[:], in_=ld_dve_d[:, :])
            ld_act_sb = constp.tile([P, n_act], f32)
            nc.sync.dma_start(out=ld_act_sb[:], in_=ld_act_d[:, :])
            normd_sb = constp.tile([P, N_TILES], f32)
            nc.sync.dma_start(out=normd_sb[:], in_=normd_d[:, :])

            for g in range(N_GROUPS):
                icols = int(icb[g].sum())
                mt_i = metap.tile([P, max(icols, 1)], i16, tag="mi")
                ic0 = int(icol_off_gb[g, 0])
                nc.sync.dma_start(out=mt_i[:, :icols], in_=idxm[:, ic0 : ic0 + icols])

                Cg = int(C_g[g])
                X_full = xp.tile([P, Cmax * P], bf16, tag="X")
                X = X_full[:, : Cg * P]
                for b in range(NB):
                    n_idx = int(nidx[g, b])
                    if n_idx == 0:
                        continue
                    co = int(co_in_g[g, b])
                    cb = int(ct[g, b])
                    io = int(icol_off_gb[g, b]) - ic0
                    icbb = int(icb[g, b])
                    if n_idx % P:
                        nc.scalar.memzero(X[:, (co + cb - 1) * P : (co + cb) * P])
                    nc.gpsimd.dma_gather(
                        out_ap=X[:, co * P : (co + cb) * P].rearrange(
                            "p (c e) -> p c e", e=P
                        ),
                        in_ap=feats[
                            b * BUCKET_MAX : min((b + 1) * BUCKET_MAX, N_NODES), :
                        ],
                        idxs_ap=mt_i[:, io : io + icbb],
                        num_idxs=n_idx,
                        num_idxs_reg=n_idx,
                        elem_size=P,
                        single_packet=False,
                        queue_num=b % 4,
                    )

                z_ps = {}
                for run in layout["runs_per_group"][g]:
                    ops = run["ops"]
                    for o in ops:
                        if o["start"]:
                            z_ps[o["tin"]] = zpsp.tile(
                                [P, P], f32, tag="z", name=f"z{g}_{o['tin']}"
                            )
                    if run["dve"]:
                        kk = len(ops)
                        sc0 = run["scol0"]
                        S_big = sp.tile([P, RUN_K * P], bf16, tag="S")
                        nc.vector.tensor_tensor(
                            out=S_big[:, : kk * P].rearrange("p (c e) -> p c e", e=P),
                            in0=iota_b[:, :].unsqueeze(1).to_broadcast([P, kk, P]),
                            in1=ld_dve_sb[:, sc0 : sc0 + kk]
                            .unsqueeze(2)
                            .to_broadcast([P, kk, P]),
                            op=mybir.AluOpType.is_equal,
                        )
                        for j, o in enumerate(ops):
                            c = o["chunk"]
                            nc.tensor.matmul(
                                out=z_ps[o["tin"]][:],
                                lhsT=X[:, c * P : (c + 1) * P],
                                rhs=S_big[:, j * P : (j + 1) * P],
                                start=o["start"],
                                stop=o["stop"],
                            )
                    else:
                        for o in ops:
                            S = sp.tile([P, P], bf16, tag="Sa")
                            t2 = sp.tile([P, P], f32, tag="T2")
                            nc.scalar.activation(
                                out=t2[:],
                                in_=iota_f[:],
                                func=mybir.ActivationFunctionType.Square,
                                bias=ld_act_sb[:, o["scol"] : o["scol"] + 1],
                                scale=-1.0,
                            )
                            nc.scalar.activation(
                                out=S[:],
                                in_=t2[:],
                                func=mybir.ActivationFunctionType.Relu,
                                bias=1.0,
                                scale=-1.0,
                            )
                            c = o["chunk"]
                            nc.tensor.matmul(
                                out=z_ps[o["tin"]][:],
                                lhsT=X[:, c * P : (c + 1) * P],
                                rhs=S[:],
                                start=o["start"],
                                stop=o["stop"],
                            )

                for tin, tn in enumerate(gtiles[g]):
                    zT_sb = postp.tile([P, P], bf16, tag="zT")
                    nc.scalar.copy(out=zT_sb[:], in_=z_ps[tin][:])
                    y_ps = ypsp.tile([P, P], f32)
                    nc.tensor.matmul(
                        out=y_ps[:], lhsT=zT_sb[:], rhs=wt_sb[:], start=True, stop=True
                    )
                    rows = P if tn < N_TILES - 1 else ROWS_LAST
                    y_sb = postp.tile([P, P], f32, tag="y")
                    nc.scalar.activation(
                        out=y_sb[:],
                        in_=y_ps[:],
                        func=mybir.ActivationFunctionType.Relu,
                        scale=normd_sb[:, tn : tn + 1],
                    )
                    res_sb = postp.tile([P, P], f32, tag="res")
                    nc.sync.dma_start(
                        out=res_sb[:rows], in_=resid[tn * P : tn * P + rows, :]
                    )
                    o_sb = postp.tile([P, P], f32, tag="o")
                    nc.vector.tensor_add(
                        out=o_sb[:rows], in0=y_sb[:rows], in1=res_sb[:rows]
                    )
                    nc.sync.dma_start(
                        out=out[tn * P : tn * P + rows, :], in_=o_sb[:rows]
                    )
    nc.finalize()
    return nc


def _run(features, W, edge_src, edge_dst, trace=False, **spmd_kwargs):
    in_maps, layout = _prepare(features, W, edge_src, edge_dst)
    nc = _build_program(layout)
    br = run_bass_kernel_spmd(
        nc, in_maps, core_ids=list(range(N_CORES)), trace=trace, **spmd_kwargs
    )
    outs = [r["out"] for r in br.results]
    full = np.concatenate(outs, axis=0).astype(np.float32)
    return full, br


def kernel(features, W, edge_src, edge_dst):
    out, _ = _run(features, W, edge_src, edge_dst, trace=False)
    return out
